# revision 16
# baseline (speedup 1.0000x reference)
"""AutoCorrelation (FFT cross-correlation + full-sort delay aggregation) on 8 NeuronCores.

Math (per batch b, channels c = (h,e), C = 512, L = 512):
  mv[t]   = (1/C) sum_s <q[s+t], k[s]>                     (channel-mean correlation)
  rank0   = ranks of mv[batch 0]
  g[b, m] = softmax(mv[b])[i] where rank_b[i] == rank_0[m] (rank-matched scatter)
  out[b,t,c] = sum_r g[b,(r-t)%L] * v[b,r,c]               (circular correlation)

Implementation (time domain, no FFT):
  - G = q k^T Gram via bf16 matmuls of host-transposed inputs; each 128-row
    block's rhs is column-rotated by the block offset so all 64 matmuls
    accumulate into ONE PSUM tile S[p,y] = sum_j G[128j+p, (y+128j)%512].
  - mv needs the per-partition diagonal sum mv[t] = sum_p S[p,(p-t)%512]:
    doubled DRAM write + stride-1025 diagonal DMA read (reversed coords), then
    one ones-matmul that both partition-reduces and broadcasts to [128,512].
  - ranks via DVE compare+accumulate; match vs batch-0 via fp16 is_equal at 2x
    DVE rate; weights scattered by 4 tiny matmuls.
  - stage C circulant lhs from a broadcast + doubled DRAM write + stride-1023
    diagonal DMA read; aggregation is 16 bf16 matmuls per batch.
  - All mv logic runs in reversed coordinates (mv_rev[u] = mv[511-u]) so every
    DMA access pattern has positive strides; the reversal cancels in stage C.
  - Slots are software-pipelined by emission order (A=gram+roundtrip, B=ranks+
    weights, C=aggregation) so the PE stream interleaves independent slots
    while each slot's DMA roundtrips are in flight.
"""

import sys
for _p in ('/opt/trn_rl_repo',):
    if _p not in sys.path:
        sys.path.insert(0, _p)

import numpy as np
import ml_dtypes
from contextlib import ExitStack

import concourse.bass as bass
import concourse.bacc as bacc
import concourse.tile as tile
import concourse.mybir as mybir
from concourse.bass_utils import run_bass_kernel_spmd

F32 = mybir.dt.float32
F32R = mybir.dt.float32r
BF16 = mybir.dt.bfloat16
F16 = mybir.dt.float16
AL = mybir.AluOpType
AF = mybir.ActivationFunctionType

B, L, H, E = 32, 512, 8, 64
C = H * E          # 512 channels per batch
NCORES = 8
NB = B // NCORES   # 4 local batches per core
NSLOT = NB + 1     # + redundant batch-0 slot (slot index NB)

_NC_CACHE = None


def _build():
    global _NC_CACHE
    if _NC_CACHE is not None:
        return _NC_CACHE

    nc = bacc.Bacc("TRN2", target_bir_lowering=False, debug=False, num_devices=NCORES)
    tc = tile.TileContext(nc)

    qT_all = nc.dram_tensor("qT_all", [NB, C, L], BF16, kind="ExternalInput")
    kT_all = nc.dram_tensor("kT_all", [NB, C, L], BF16, kind="ExternalInput")
    n0h_d = nc.dram_tensor("n0h_in", [1, L], F16, kind="ExternalInput")
    v_all = nc.dram_tensor("v_all", [NB, L, C], BF16, kind="ExternalInput")
    out_all = nc.dram_tensor("out_all", [NB, L, C], BF16, kind="ExternalOutput")

    onesB16_np = np.full((128, 128), 1.0 / L, np.float32).astype(np.float16)
    ones1_np = np.full((128, 1), 1.0, np.float32)
    one1_np = np.ones((1, 1), np.float32)

    onesB16_d = nc.inline_tensor(onesB16_np, "onesB16")
    ones1_d = nc.inline_tensor(ones1_np, "ones1")
    one1_d = nc.inline_tensor(one1_np, "one1")

    with tc, ExitStack() as ctx:
        cpool = ctx.enter_context(tc.tile_pool(name="consts", bufs=1))
        iopool = ctx.enter_context(tc.tile_pool(name="io", bufs=1))
        wpool = ctx.enter_context(tc.tile_pool(name="work", bufs=1))
        pbig = ctx.enter_context(tc.tile_pool(name="pbig", bufs=1, space="PSUM"))
        psmall = ctx.enter_context(tc.tile_pool(name="psmall", bufs=1, space="PSUM"))
        dpool = ctx.enter_context(tc.tile_pool(name="dscratch", bufs=1, space="DRAM"))

        onesB16 = cpool.tile([128, 128], F16, name="onesB16")
        nc.sync.dma_start(onesB16[:], onesB16_d.ap())
        one1 = cpool.tile([1, 1], F32, name="one1")
        nc.sync.dma_start(one1[:], one1_d.ap())

        st = {}  # per-slot state

        def load_qk(s):
            tiles = []
            for (dram, tag) in ((qT_all, "qt"), (kT_all, "kt")):
                t = iopool.tile([128, 2048], BF16, name=f"{tag}_s{s}", tag=tag, bufs=2)
                src = bass.AP(tensor=dram, offset=s * C * L,
                              ap=[[512, 128], [65536, 4], [1, 512]])
                dst = bass.AP(tensor=t[:].tensor, offset=t[:].offset,
                              ap=[[t[:].ap[0][0], 128], [512, 4], [1, 512]])
                nc.sync.dma_start(dst, src)
                tiles.append(t)
            return tiles

        def phase_A(s):
            """Loads, Gram-accumulate S, diagonal roundtrip launch."""
            qt, kt = load_qk(s)
            S_ps = pbig.tile([128, 512], F32, name=f"S_s{s}", tag="Sps", bufs=2)
            for r in range(4):
                n = 0
                for j in range(4):
                    for cc in range(4):
                        nc.tensor.matmul(
                            S_ps[:, 128 * r:128 * (r + 1)],
                            qt[:, 512 * cc + 128 * j: 512 * cc + 128 * (j + 1)],
                            kt[:, 512 * cc + 128 * ((r + j) % 4): 512 * cc + 128 * ((r + j) % 4 + 1)],
                            start=(n == 0), stop=(n == 15))
                        n += 1
            S_sb = wpool.tile([128, 512], F16, name=f"Ssb_s{s}", tag="Ssb", bufs=2)
            nc.scalar.copy(S_sb[:], S_ps[:])
            Sd = dpool.tile([128, 1024], F16, name=f"Sd_s{s}", tag="Sd", bufs=2)
            src = bass.AP(tensor=S_sb[:].tensor, offset=S_sb[:].offset,
                          ap=[[S_sb[:].ap[0][0], 128], [0, 2], [1, 512]])
            dst = bass.AP(tensor=Sd[:].tensor, offset=0, ap=[[1024, 128], [512, 2], [1, 512]])
            nc.sync.dma_start(dst, src)
            # S''[p, u] = S[p, (p + u + 1) % 512]  (diagonal, reversed coords)
            Spp = wpool.tile([128, 512], F16, name=f"Spp_s{s}", tag="Spp", bufs=2)
            nc.sync.dma_start(Spp[:], bass.AP(tensor=Sd[:].tensor, offset=1,
                                              ap=[[1025, 128], [1, 512]]))
            st[s] = {"Spp": Spp}

        def phase_Bhead(s):
            """mv broadcast + transposes (shared by all slots)."""
            Spp = st[s]["Spp"]
            mvB_ps = pbig.tile([128, 512], F32, name=f"mvB_s{s}", tag="mvB", bufs=2)
            nc.tensor.matmul(mvB_ps[:], onesB16[:], Spp[:], start=True, stop=True)
            mv_sb = wpool.tile([1, 512], F32, name=f"mvsb_s{s}", tag="mvsb", bufs=2)
            nc.scalar.copy(mv_sb[:], mvB_ps[0:1, :])
            mvT_ps = psmall.tile([128, 4], F32, name=f"mvT_s{s}", tag="psm", bufs=2)
            for j in range(4):
                nc.tensor.transpose(mvT_ps[:, j:j + 1], mv_sb[0:1, 128 * j:128 * (j + 1)], one1[:])
            mvT = wpool.tile([128, 4], F32, name=f"mvTsb_s{s}", tag="mvTsb", bufs=2)
            nc.scalar.copy(mvT[:], mvT_ps[:])
            st[s].update(mvB_ps=mvB_ps, mv_sb=mv_sb, mvT_ps=mvT_ps, mvT=mvT)

        def phase_B0():
            """Load the replicated batch-0 rank row and broadcast it."""
            n0h = wpool.tile([1, 512], F16, name="n0h", bufs=1)
            nc.sync.dma_start(n0h[:], n0h_d.ap())
            n0hB = cpool.tile([128, 512], F16, name="n0hB")
            nc.gpsimd.partition_broadcast(n0hB[:], n0h[:])
            st["n0hB"] = n0hB

        def phase_B(s):
            """Ranks, softmax, rank-match scatter, circulant lhs roundtrip launch."""
            phase_Bhead(s)
            d = st[s]
            n0hB = st["n0hB"]
            rs = wpool.tile([128, 4], F32, name=f"rs_{s}", tag="rs", bufs=2)
            for j in range(4):
                c2 = wpool.tile([128, 512], F32R, name=f"c2_{s}_{j}", tag="c2", bufs=2)
                nc.vector.tensor_scalar(c2[:], d["mvB_ps"][:], d["mvT"][:, j:j + 1], 0.0, AL.is_lt,
                                        AL.add, accum_out=rs[:, j:j + 1])
            rsm = wpool.tile([128, 4], F32, name=f"rsm_{s}", tag="rsm", bufs=2)
            nc.vector.tensor_scalar(rsm[:], rs[:], -255.5, None, AL.add)

            smc = wpool.tile([128, 4], BF16, name=f"smc_{s}", tag="smc", bufs=2)
            nc.scalar.activation(smc[:], d["mvT_ps"][:], AF.Exp)
            expz = wpool.tile([1, 512], F32, name=f"expz_{s}", tag="expz", bufs=2)
            z_sb = wpool.tile([1, 1], F32, name=f"z_{s}", tag="z", bufs=2)
            nc.scalar.activation(expz[:], d["mv_sb"][:], AF.Exp, accum_out=z_sb[:])
            rz = wpool.tile([1, 1], F32, name=f"rz_{s}", tag="rz", bufs=2)
            nc.vector.reciprocal(rz[:], z_sb[:])

            g_ps = psmall.tile([1, 512], F32, name=f"gps_{s}", tag="psm", bufs=2)
            for j in range(4):
                wt = wpool.tile([128, 512], BF16, name=f"wt_{s}_{j}", tag="wt", bufs=2)
                nc.vector.tensor_scalar(wt[:], n0hB[:], rsm[:, j:j + 1], None, AL.is_equal)
                nc.tensor.matmul(g_ps[:], smc[:, j:j + 1], wt[:], start=(j == 0), stop=(j == 3))
            gn = wpool.tile([1, 512], BF16, name=f"gn_{s}", tag="gn", bufs=2)
            nc.scalar.activation(gn[:], g_ps[:], AF.Copy, bias=0.0, scale=rz[:])

            gB = wpool.tile([128, 512], BF16, name=f"gB_{s}", tag="gB", bufs=2)
            nc.gpsimd.partition_broadcast(gB[:], gn[:])
            Gb = dpool.tile([128, 1024], BF16, name=f"Gb_{s}", tag="Gb", bufs=2)
            src = bass.AP(tensor=gB[:].tensor, offset=gB[:].offset,
                          ap=[[gB[:].ap[0][0], 128], [0, 2], [1, 512]])
            dst = bass.AP(tensor=Gb[:].tensor, offset=0, ap=[[1024, 128], [512, 2], [1, 512]])
            nc.sync.dma_start(dst, src)
            # T[p, 128*mm + t'] = g_rev[(511 - p + 128*mm + t') % 512]
            T = wpool.tile([128, 512], BF16, name=f"T_{s}", tag="T", bufs=2)
            srcT = bass.AP(tensor=Gb[:].tensor, offset=511,
                           ap=[[1023, 128], [128, 4], [1, 128]])
            dstT = bass.AP(tensor=T[:].tensor, offset=T[:].offset,
                           ap=[[T[:].ap[0][0], 128], [128, 4], [1, 128]])
            nc.sync.dma_start(dstT, srcT)
            # v load for stage C
            vt = iopool.tile([128, 2048], BF16, name=f"v_s{s}", tag="vt", bufs=2)
            src = bass.AP(tensor=v_all, offset=s * L * C,
                          ap=[[512, 128], [65536, 4], [1, 512]])
            dst = bass.AP(tensor=vt[:].tensor, offset=vt[:].offset,
                          ap=[[vt[:].ap[0][0], 128], [512, 4], [1, 512]])
            nc.sync.dma_start(dst, src)
            st[s].update(T=T, vt=vt)

        def phase_C(s):
            """Circulant aggregation + output."""
            T, vt = st[s]["T"], st[s]["vt"]
            osb = wpool.tile([128, 2048], BF16, name=f"osb_{s}", tag="osb", bufs=2)
            for tt in range(4):
                o_ps = pbig.tile([128, 512], F32, name=f"ops_{s}_{tt}", tag="ops", bufs=2)
                for ss in range(4):
                    mm = (tt - ss) % 4
                    nc.tensor.matmul(o_ps[:], T[:, 128 * mm:128 * (mm + 1)],
                                     vt[:, 512 * ss:512 * (ss + 1)],
                                     start=(ss == 0), stop=(ss == 3))
                if tt % 2 == 0:
                    nc.scalar.copy(osb[:, 512 * tt:512 * (tt + 1)], o_ps[:])
                else:
                    nc.vector.tensor_copy(osb[:, 512 * tt:512 * (tt + 1)], o_ps[:])
                dsto = bass.AP(tensor=out_all, offset=s * L * C + tt * 128 * 512,
                               ap=[[512, 128], [1, 512]])
                srco = bass.AP(tensor=osb[:].tensor, offset=osb[:].offset + tt * 512,
                               ap=[[osb[:].ap[0][0], 128], [1, 512]])
                nc.sync.dma_start(dsto, srco)

        # software-pipelined emission
        phase_B0()
        phase_A(0)
        phase_A(1)
        phase_B(0)
        phase_A(2)
        phase_C(0)
        phase_B(1)
        phase_A(3)
        phase_C(1)
        phase_B(2)
        phase_C(2)
        phase_B(3)
        phase_C(3)

    nc.compile()
    _NC_CACHE = nc
    return nc


def kernel(queries, keys, values):
    q = np.ascontiguousarray(queries, dtype=np.float32).reshape(B, L, C)
    k = np.ascontiguousarray(keys, dtype=np.float32).reshape(B, L, C)
    v = np.ascontiguousarray(values, dtype=np.float32).reshape(B, L, C)
    qT = np.ascontiguousarray(q.transpose(0, 2, 1)).astype(ml_dtypes.bfloat16)
    kT = np.ascontiguousarray(k.transpose(0, 2, 1)).astype(ml_dtypes.bfloat16)
    vb = v.astype(ml_dtypes.bfloat16)
    # batch-0 delay ranks ("replicate that row" sharding): mv0 once via FFT,
    # rank row in the kernel's reversed coordinates, replicated to every core.
    q0f = np.fft.rfft(q[0], axis=0)          # [F, C] over time axis
    k0f = np.fft.rfft(k[0], axis=0)
    corr0 = np.fft.irfft(q0f * np.conj(k0f), n=L, axis=0)  # [L, C]
    mv0 = corr0.mean(axis=1)                 # [L]
    nless0 = (mv0[None, :] < mv0[:, None]).sum(axis=1).astype(np.float32)
    n0h_rev = (nless0[::-1] - 255.5).astype(np.float16)[None, :]
    nc = _build()
    in_maps = []
    for c in range(NCORES):
        sl = slice(NB * c, NB * (c + 1))
        in_maps.append({
            "qT_all": qT[sl],
            "kT_all": kT[sl],
            "v_all": vb[sl],
            "n0h_in": n0h_rev,
        })
    res = run_bass_kernel_spmd(nc, in_maps, core_ids=list(range(NCORES)))
    out = np.concatenate([res.results[c]["out_all"].astype(np.float32) for c in range(NCORES)], axis=0)
    return out.reshape(B, L, H, E)


if __name__ == "__main__":
    rng = np.random.default_rng(0)
    qq = rng.standard_normal((B, L, H, E)).astype(np.float32)
    kk = rng.standard_normal((B, L, H, E)).astype(np.float32)
    vv = rng.standard_normal((B, L, H, E)).astype(np.float32)
    o = kernel(queries=qq, keys=kk, values=vv)
    print(o.shape, o.dtype, np.abs(o).max())


# revision 18
# speedup vs baseline: 1.0350x; 1.0350x over previous
"""AutoCorrelation (FFT cross-correlation + full-sort delay aggregation) on 8 NeuronCores.

Math (per batch b, channels c = (h,e), C = 512, L = 512):
  mv[t]   = (1/C) sum_s <q[s+t], k[s]>                     (channel-mean correlation)
  rank0   = ranks of mv[batch 0]
  g[b, m] = softmax(mv[b])[i] where rank_b[i] == rank_0[m] (rank-matched scatter)
  out[b,t,c] = sum_r g[b,(r-t)%L] * v[b,r,c]               (circular correlation)

Implementation (time domain, no FFT):
  - G = q k^T Gram via bf16 matmuls of host-transposed inputs; each 128-row
    block's rhs is column-rotated by the block offset so all 64 matmuls
    accumulate into ONE PSUM tile S[p,y] = sum_j G[128j+p, (y+128j)%512].
  - mv needs the per-partition diagonal sum mv[t] = sum_p S[p,(p-t)%512]:
    doubled DRAM write + stride-1025 diagonal DMA read (reversed coords), then
    one ones-matmul that both partition-reduces and broadcasts to [128,512].
  - ranks via DVE compare+accumulate; match vs batch-0 via fp16 is_equal at 2x
    DVE rate; weights scattered by 4 tiny matmuls.
  - stage C circulant lhs from a broadcast + doubled DRAM write + stride-1023
    diagonal DMA read; aggregation is 16 bf16 matmuls per batch.
  - All mv logic runs in reversed coordinates (mv_rev[u] = mv[511-u]) so every
    DMA access pattern has positive strides; the reversal cancels in stage C.
  - Slots are software-pipelined by emission order (A=gram+roundtrip, B=ranks+
    weights, C=aggregation) so the PE stream interleaves independent slots
    while each slot's DMA roundtrips are in flight.
"""

import sys
for _p in ('/opt/trn_rl_repo',):
    if _p not in sys.path:
        sys.path.insert(0, _p)

import numpy as np
import ml_dtypes
from contextlib import ExitStack

import concourse.bass as bass
import concourse.bacc as bacc
import concourse.tile as tile
import concourse.mybir as mybir
from concourse.bass_utils import run_bass_kernel_spmd

F32 = mybir.dt.float32
F32R = mybir.dt.float32r
BF16 = mybir.dt.bfloat16
F16 = mybir.dt.float16
AL = mybir.AluOpType
AF = mybir.ActivationFunctionType

B, L, H, E = 32, 512, 8, 64
C = H * E          # 512 channels per batch
NCORES = 8
NB = B // NCORES   # 4 local batches per core
NSLOT = NB + 1     # + redundant batch-0 slot (slot index NB)

_NC_CACHE = None


def _build():
    global _NC_CACHE
    if _NC_CACHE is not None:
        return _NC_CACHE

    nc = bacc.Bacc("TRN2", target_bir_lowering=False, debug=False, num_devices=NCORES)
    tc = tile.TileContext(nc)

    qT_all = nc.dram_tensor("qT_all", [NB, C, L], BF16, kind="ExternalInput")
    kT_all = nc.dram_tensor("kT_all", [NB, C, L], BF16, kind="ExternalInput")
    n0h_d = nc.dram_tensor("n0h_in", [1, L], F16, kind="ExternalInput")
    v_all = nc.dram_tensor("v_all", [NB, L, C], BF16, kind="ExternalInput")
    out_all = nc.dram_tensor("out_all", [NB, L, C], BF16, kind="ExternalOutput")

    onesB16_np = np.full((128, 128), 1.0 / L, np.float32).astype(np.float16)
    ones1_np = np.full((128, 1), 1.0, np.float32)
    one1_np = np.ones((1, 1), np.float32)

    onesB16_d = nc.inline_tensor(onesB16_np, "onesB16")
    ones1_d = nc.inline_tensor(ones1_np, "ones1")
    one1_d = nc.inline_tensor(one1_np, "one1")

    with tc, ExitStack() as ctx:
        cpool = ctx.enter_context(tc.tile_pool(name="consts", bufs=1))
        iopool = ctx.enter_context(tc.tile_pool(name="io", bufs=1))
        wpool = ctx.enter_context(tc.tile_pool(name="work", bufs=1))
        pbig = ctx.enter_context(tc.tile_pool(name="pbig", bufs=1, space="PSUM"))
        psmall = ctx.enter_context(tc.tile_pool(name="psmall", bufs=1, space="PSUM"))
        dpool = ctx.enter_context(tc.tile_pool(name="dscratch", bufs=1, space="DRAM"))

        onesB16 = cpool.tile([128, 128], F16, name="onesB16")
        nc.sync.dma_start(onesB16[:], onesB16_d.ap())
        one1 = cpool.tile([1, 1], F32, name="one1")
        nc.sync.dma_start(one1[:], one1_d.ap())

        st = {}  # per-slot state

        def load_qk(s):
            tiles = []
            for (dram, tag) in ((qT_all, "qt"), (kT_all, "kt")):
                t = iopool.tile([128, 2048], BF16, name=f"{tag}_s{s}", tag=tag, bufs=2)
                src = bass.AP(tensor=dram, offset=s * C * L,
                              ap=[[512, 128], [65536, 4], [1, 512]])
                dst = bass.AP(tensor=t[:].tensor, offset=t[:].offset,
                              ap=[[t[:].ap[0][0], 128], [512, 4], [1, 512]])
                nc.sync.dma_start(dst, src)
                tiles.append(t)
            return tiles

        def phase_A(s):
            """Loads, Gram-accumulate S, diagonal roundtrip launch."""
            qt, kt = load_qk(s)
            S_ps = pbig.tile([128, 512], F32, name=f"S_s{s}", tag="Sps", bufs=2)
            for r in range(4):
                n = 0
                for j in range(4):
                    for cc in range(4):
                        nc.tensor.matmul(
                            S_ps[:, 128 * r:128 * (r + 1)],
                            qt[:, 512 * cc + 128 * j: 512 * cc + 128 * (j + 1)],
                            kt[:, 512 * cc + 128 * ((r + j) % 4): 512 * cc + 128 * ((r + j) % 4 + 1)],
                            start=(n == 0), stop=(n == 15))
                        n += 1
            S_sb = wpool.tile([128, 512], F16, name=f"Ssb_s{s}", tag="Ssb", bufs=2)
            nc.scalar.copy(S_sb[:], S_ps[:])
            Sd = dpool.tile([128, 1024], F16, name=f"Sd_s{s}", tag="Sd", bufs=2)
            src = bass.AP(tensor=S_sb[:].tensor, offset=S_sb[:].offset,
                          ap=[[S_sb[:].ap[0][0], 128], [0, 2], [1, 512]])
            dst = bass.AP(tensor=Sd[:].tensor, offset=0, ap=[[1024, 128], [512, 2], [1, 512]])
            nc.sync.dma_start(dst, src)
            # S''[p, u] = S[p, (p + u + 1) % 512]  (diagonal, reversed coords)
            Spp = wpool.tile([128, 512], F16, name=f"Spp_s{s}", tag="Spp", bufs=2)
            nc.sync.dma_start(Spp[:], bass.AP(tensor=Sd[:].tensor, offset=1,
                                              ap=[[1025, 128], [1, 512]]))
            st[s] = {"Spp": Spp}

        def phase_Bhead(s):
            """mv broadcast + transposes (shared by all slots)."""
            Spp = st[s]["Spp"]
            mvB_ps = pbig.tile([128, 512], F32, name=f"mvB_s{s}", tag="mvB", bufs=2)
            nc.tensor.matmul(mvB_ps[:], onesB16[:], Spp[:], start=True, stop=True)
            mv_sb = wpool.tile([1, 512], F32, name=f"mvsb_s{s}", tag="mvsb", bufs=2)
            nc.scalar.copy(mv_sb[:], mvB_ps[0:1, :])
            mvT_ps = psmall.tile([128, 4], F32, name=f"mvT_s{s}", tag="psm", bufs=2)
            for j in range(4):
                nc.tensor.transpose(mvT_ps[:, j:j + 1], mv_sb[0:1, 128 * j:128 * (j + 1)], one1[:])
            mvT = wpool.tile([128, 4], F32, name=f"mvTsb_s{s}", tag="mvTsb", bufs=2)
            nc.scalar.copy(mvT[:], mvT_ps[:])
            st[s].update(mvB_ps=mvB_ps, mv_sb=mv_sb, mvT_ps=mvT_ps, mvT=mvT)

        def phase_B0():
            """Load the replicated batch-0 rank row and broadcast it."""
            n0h = wpool.tile([1, 512], F16, name="n0h", bufs=1)
            nc.sync.dma_start(n0h[:], n0h_d.ap())
            n0hB = cpool.tile([128, 512], F16, name="n0hB")
            nc.gpsimd.partition_broadcast(n0hB[:], n0h[:])
            st["n0hB"] = n0hB

        def phase_B(s):
            """Ranks, softmax, rank-match scatter, circulant lhs roundtrip launch."""
            phase_Bhead(s)
            d = st[s]
            n0hB = st["n0hB"]
            rs = wpool.tile([128, 4], F32, name=f"rs_{s}", tag="rs", bufs=2)
            for j in range(4):
                c2 = wpool.tile([128, 512], F32R, name=f"c2_{s}_{j}", tag="c2", bufs=2)
                nc.vector.tensor_scalar(c2[:], d["mvB_ps"][:], d["mvT"][:, j:j + 1], 0.0, AL.is_lt,
                                        AL.add, accum_out=rs[:, j:j + 1])
            rsm = wpool.tile([128, 4], F32, name=f"rsm_{s}", tag="rsm", bufs=2)
            nc.vector.tensor_scalar(rsm[:], rs[:], -255.5, None, AL.add)

            smc = wpool.tile([128, 4], BF16, name=f"smc_{s}", tag="smc", bufs=2)
            nc.scalar.activation(smc[:], d["mvT_ps"][:], AF.Exp)
            expz = wpool.tile([1, 512], F32, name=f"expz_{s}", tag="expz", bufs=2)
            z_sb = wpool.tile([1, 1], F32, name=f"z_{s}", tag="z", bufs=2)
            nc.scalar.activation(expz[:], d["mv_sb"][:], AF.Exp, accum_out=z_sb[:])
            rz = wpool.tile([1, 1], F32, name=f"rz_{s}", tag="rz", bufs=2)
            nc.vector.reciprocal(rz[:], z_sb[:])

            g_ps = psmall.tile([1, 512], F32, name=f"gps_{s}", tag="psm", bufs=2)
            for j in range(4):
                wt = wpool.tile([128, 512], BF16, name=f"wt_{s}_{j}", tag="wt", bufs=2)
                nc.vector.tensor_scalar(wt[:], n0hB[:], rsm[:, j:j + 1], None, AL.is_equal)
                nc.tensor.matmul(g_ps[:], smc[:, j:j + 1], wt[:], start=(j == 0), stop=(j == 3))
            gn = wpool.tile([1, 512], BF16, name=f"gn_{s}", tag="gn", bufs=2)
            nc.scalar.activation(gn[:], g_ps[:], AF.Copy, bias=0.0, scale=rz[:])

            gB = wpool.tile([128, 512], BF16, name=f"gB_{s}", tag="gB", bufs=2)
            nc.gpsimd.partition_broadcast(gB[:], gn[:])
            Gb = dpool.tile([128, 1024], BF16, name=f"Gb_{s}", tag="Gb", bufs=2)
            src = bass.AP(tensor=gB[:].tensor, offset=gB[:].offset,
                          ap=[[gB[:].ap[0][0], 128], [0, 2], [1, 512]])
            dst = bass.AP(tensor=Gb[:].tensor, offset=0, ap=[[1024, 128], [512, 2], [1, 512]])
            nc.sync.dma_start(dst, src)
            # T[p, 128*mm + t'] = g_rev[(511 - p + 128*mm + t') % 512]
            T = wpool.tile([128, 512], BF16, name=f"T_{s}", tag="T", bufs=2)
            srcT = bass.AP(tensor=Gb[:].tensor, offset=511,
                           ap=[[1023, 128], [128, 4], [1, 128]])
            dstT = bass.AP(tensor=T[:].tensor, offset=T[:].offset,
                           ap=[[T[:].ap[0][0], 128], [128, 4], [1, 128]])
            nc.sync.dma_start(dstT, srcT)
            # v load for stage C
            vt = iopool.tile([128, 2048], BF16, name=f"v_s{s}", tag="vt", bufs=2)
            src = bass.AP(tensor=v_all, offset=s * L * C,
                          ap=[[512, 128], [65536, 4], [1, 512]])
            dst = bass.AP(tensor=vt[:].tensor, offset=vt[:].offset,
                          ap=[[vt[:].ap[0][0], 128], [512, 4], [1, 512]])
            nc.sync.dma_start(dst, src)
            st[s].update(T=T, vt=vt)

        def phase_C(s):
            """Circulant aggregation + output."""
            T, vt = st[s]["T"], st[s]["vt"]
            osb = wpool.tile([128, 2048], BF16, name=f"osb_{s}", tag="osb", bufs=2)
            for tt in range(4):
                o_ps = pbig.tile([128, 512], F32, name=f"ops_{s}_{tt}", tag="ops", bufs=2)
                for ss in range(4):
                    mm = (tt - ss) % 4
                    nc.tensor.matmul(o_ps[:], T[:, 128 * mm:128 * (mm + 1)],
                                     vt[:, 512 * ss:512 * (ss + 1)],
                                     start=(ss == 0), stop=(ss == 3))
                if tt % 2 == 0:
                    nc.scalar.copy(osb[:, 512 * tt:512 * (tt + 1)], o_ps[:])
                else:
                    nc.vector.tensor_copy(osb[:, 512 * tt:512 * (tt + 1)], o_ps[:])
                dsto = bass.AP(tensor=out_all, offset=s * L * C + tt * 128 * 512,
                               ap=[[512, 128], [1, 512]])
                srco = bass.AP(tensor=osb[:].tensor, offset=osb[:].offset + tt * 512,
                               ap=[[osb[:].ap[0][0], 128], [1, 512]])
                nc.sync.dma_start(dsto, srco)

        # software-pipelined emission
        import os as _os
        _order = _os.environ.get("EMIT", "Z;A0;B0;A1;B1;A2;B2;C0;A3;B3;C1;C2;C3")
        for tok in _order.split(";"):
            if tok == "Z":
                phase_B0()
            elif tok[0] == "A":
                phase_A(int(tok[1]))
            elif tok[0] == "B":
                phase_B(int(tok[1]))
            elif tok[0] == "C":
                phase_C(int(tok[1]))

    nc.compile()
    _NC_CACHE = nc
    return nc


def kernel(queries, keys, values):
    q = np.ascontiguousarray(queries, dtype=np.float32).reshape(B, L, C)
    k = np.ascontiguousarray(keys, dtype=np.float32).reshape(B, L, C)
    v = np.ascontiguousarray(values, dtype=np.float32).reshape(B, L, C)
    qT = np.ascontiguousarray(q.transpose(0, 2, 1)).astype(ml_dtypes.bfloat16)
    kT = np.ascontiguousarray(k.transpose(0, 2, 1)).astype(ml_dtypes.bfloat16)
    vb = v.astype(ml_dtypes.bfloat16)
    # batch-0 delay ranks ("replicate that row" sharding): mv0 once via FFT,
    # rank row in the kernel's reversed coordinates, replicated to every core.
    q0f = np.fft.rfft(q[0], axis=0)          # [F, C] over time axis
    k0f = np.fft.rfft(k[0], axis=0)
    corr0 = np.fft.irfft(q0f * np.conj(k0f), n=L, axis=0)  # [L, C]
    mv0 = corr0.mean(axis=1)                 # [L]
    nless0 = (mv0[None, :] < mv0[:, None]).sum(axis=1).astype(np.float32)
    n0h_rev = (nless0[::-1] - 255.5).astype(np.float16)[None, :]
    nc = _build()
    in_maps = []
    for c in range(NCORES):
        sl = slice(NB * c, NB * (c + 1))
        in_maps.append({
            "qT_all": qT[sl],
            "kT_all": kT[sl],
            "v_all": vb[sl],
            "n0h_in": n0h_rev,
        })
    res = run_bass_kernel_spmd(nc, in_maps, core_ids=list(range(NCORES)))
    out = np.concatenate([res.results[c]["out_all"].astype(np.float32) for c in range(NCORES)], axis=0)
    return out.reshape(B, L, H, E)


if __name__ == "__main__":
    rng = np.random.default_rng(0)
    qq = rng.standard_normal((B, L, H, E)).astype(np.float32)
    kk = rng.standard_normal((B, L, H, E)).astype(np.float32)
    vv = rng.standard_normal((B, L, H, E)).astype(np.float32)
    o = kernel(queries=qq, keys=kk, values=vv)
    print(o.shape, o.dtype, np.abs(o).max())


# revision 19
# speedup vs baseline: 1.1065x; 1.0691x over previous
"""AutoCorrelation (FFT cross-correlation + full-sort delay aggregation) on 8 NeuronCores.

Math (per batch b, channels c = (h,e), C = 512, L = 512):
  mv[t]   = (1/C) sum_s <q[s+t], k[s]>                     (channel-mean correlation)
  rank0   = ranks of mv[batch 0]
  g[b, m] = softmax(mv[b])[i] where rank_b[i] == rank_0[m] (rank-matched scatter)
  out[b,t,c] = sum_r g[b,(r-t)%L] * v[b,r,c]               (circular correlation)

Implementation (time domain, no FFT):
  - G = q k^T Gram via bf16 matmuls of host-transposed inputs; each 128-row
    block's rhs is column-rotated by the block offset so all 64 matmuls
    accumulate into ONE PSUM tile S[p,y] = sum_j G[128j+p, (y+128j)%512].
  - mv needs the per-partition diagonal sum mv[t] = sum_p S[p,(p-t)%512]:
    doubled DRAM write + stride-1025 diagonal DMA read (reversed coords), then
    one ones-matmul that both partition-reduces and broadcasts to [128,512].
  - ranks via DVE compare+accumulate; match vs batch-0 via fp16 is_equal at 2x
    DVE rate; weights scattered by 4 tiny matmuls.
  - stage C circulant lhs from a broadcast + doubled DRAM write + stride-1023
    diagonal DMA read; aggregation is 16 bf16 matmuls per batch.
  - All mv logic runs in reversed coordinates (mv_rev[u] = mv[511-u]) so every
    DMA access pattern has positive strides; the reversal cancels in stage C.
  - Slots are software-pipelined by emission order (A=gram+roundtrip, B=ranks+
    weights, C=aggregation) so the PE stream interleaves independent slots
    while each slot's DMA roundtrips are in flight.
"""

import sys
for _p in ('/opt/trn_rl_repo',):
    if _p not in sys.path:
        sys.path.insert(0, _p)

import numpy as np
import ml_dtypes
from contextlib import ExitStack

import concourse.bass as bass
import concourse.bacc as bacc
import concourse.tile as tile
import concourse.mybir as mybir
from concourse.bass_utils import run_bass_kernel_spmd

F32 = mybir.dt.float32
F32R = mybir.dt.float32r
BF16 = mybir.dt.bfloat16
F16 = mybir.dt.float16
AL = mybir.AluOpType
AF = mybir.ActivationFunctionType

B, L, H, E = 32, 512, 8, 64
C = H * E          # 512 channels per batch
NCORES = 8
NB = B // NCORES   # 4 local batches per core
NSLOT = NB + 1     # + redundant batch-0 slot (slot index NB)

_NC_CACHE = None


def _build():
    global _NC_CACHE
    if _NC_CACHE is not None:
        return _NC_CACHE

    nc = bacc.Bacc("TRN2", target_bir_lowering=False, debug=False, num_devices=NCORES)
    tc = tile.TileContext(nc)

    qT_all = nc.dram_tensor("qT_all", [NB, C, L], BF16, kind="ExternalInput")
    kT_all = nc.dram_tensor("kT_all", [NB, C, L], BF16, kind="ExternalInput")
    n0h_d = nc.dram_tensor("n0h_in", [1, L], F16, kind="ExternalInput")
    v_all = nc.dram_tensor("v_all", [NB, L, C], BF16, kind="ExternalInput")
    out_all = nc.dram_tensor("out_all", [NB, L, C], BF16, kind="ExternalOutput")

    onesB16_np = np.full((128, 128), 1.0 / L, np.float32).astype(np.float16)
    ones1_np = np.full((128, 1), 1.0, np.float32)
    one1_np = np.ones((1, 1), np.float32)

    onesB16_d = nc.inline_tensor(onesB16_np, "onesB16")
    ones1_d = nc.inline_tensor(ones1_np, "ones1")
    one1_d = nc.inline_tensor(one1_np, "one1")

    with tc, ExitStack() as ctx:
        cpool = ctx.enter_context(tc.tile_pool(name="consts", bufs=1))
        iopool = ctx.enter_context(tc.tile_pool(name="io", bufs=1))
        wpool = ctx.enter_context(tc.tile_pool(name="work", bufs=1))
        pbig = ctx.enter_context(tc.tile_pool(name="pbig", bufs=1, space="PSUM"))
        psmall = ctx.enter_context(tc.tile_pool(name="psmall", bufs=1, space="PSUM"))
        dpool = ctx.enter_context(tc.tile_pool(name="dscratch", bufs=1, space="DRAM"))

        onesB16 = cpool.tile([128, 128], F16, name="onesB16")
        nc.sync.dma_start(onesB16[:], onesB16_d.ap())
        one1 = cpool.tile([1, 1], F32, name="one1")
        nc.sync.dma_start(one1[:], one1_d.ap())

        st = {}  # per-slot state

        def load_qk(s):
            tiles = []
            for (dram, tag) in ((qT_all, "qt"), (kT_all, "kt")):
                t = iopool.tile([128, 2048], BF16, name=f"{tag}_s{s}", tag=tag, bufs=2)
                src = bass.AP(tensor=dram, offset=s * C * L,
                              ap=[[512, 128], [65536, 4], [1, 512]])
                dst = bass.AP(tensor=t[:].tensor, offset=t[:].offset,
                              ap=[[t[:].ap[0][0], 128], [512, 4], [1, 512]])
                nc.sync.dma_start(dst, src)
                tiles.append(t)
            return tiles

        def phase_A(s):
            """Loads, Gram-accumulate S, diagonal roundtrip launch."""
            qt, kt = load_qk(s)
            S_ps = pbig.tile([128, 512], F32, name=f"S_s{s}", tag="Sps", bufs=2)
            for r in range(4):
                n = 0
                for j in range(4):
                    for cc in range(4):
                        nc.tensor.matmul(
                            S_ps[:, 128 * r:128 * (r + 1)],
                            qt[:, 512 * cc + 128 * j: 512 * cc + 128 * (j + 1)],
                            kt[:, 512 * cc + 128 * ((r + j) % 4): 512 * cc + 128 * ((r + j) % 4 + 1)],
                            start=(n == 0), stop=(n == 15))
                        n += 1
            S_sb = wpool.tile([128, 512], F16, name=f"Ssb_s{s}", tag="Ssb", bufs=2)
            nc.scalar.copy(S_sb[:], S_ps[:])
            Sd = dpool.tile([128, 1024], F16, name=f"Sd_s{s}", tag="Sd", bufs=2)
            src = bass.AP(tensor=S_sb[:].tensor, offset=S_sb[:].offset,
                          ap=[[S_sb[:].ap[0][0], 128], [0, 2], [1, 512]])
            dst = bass.AP(tensor=Sd[:].tensor, offset=0, ap=[[1024, 128], [512, 2], [1, 512]])
            nc.sync.dma_start(dst, src)
            # S''[p, u] = S[p, (p + u + 1) % 512]  (diagonal, reversed coords)
            Spp = wpool.tile([128, 512], F16, name=f"Spp_s{s}", tag="Spp", bufs=2)
            nc.sync.dma_start(Spp[:], bass.AP(tensor=Sd[:].tensor, offset=1,
                                              ap=[[1025, 128], [1, 512]]))
            st[s] = {"Spp": Spp}

        def phase_Bhead(s):
            """mv broadcast + transposes (shared by all slots)."""
            Spp = st[s]["Spp"]
            mvB_ps = pbig.tile([128, 512], F32, name=f"mvB_s{s}", tag="mvB", bufs=2)
            nc.tensor.matmul(mvB_ps[:], onesB16[:], Spp[:], start=True, stop=True)
            mv_sb = wpool.tile([1, 512], F32, name=f"mvsb_s{s}", tag="mvsb", bufs=2)
            nc.scalar.copy(mv_sb[:], mvB_ps[0:1, :])
            mvT_ps = psmall.tile([128, 4], F32, name=f"mvT_s{s}", tag="psm", bufs=2)
            for j in range(4):
                nc.tensor.transpose(mvT_ps[:, j:j + 1], mv_sb[0:1, 128 * j:128 * (j + 1)], one1[:])
            mvT = wpool.tile([128, 4], F32, name=f"mvTsb_s{s}", tag="mvTsb", bufs=2)
            nc.scalar.copy(mvT[:], mvT_ps[:])
            st[s].update(mvB_ps=mvB_ps, mv_sb=mv_sb, mvT_ps=mvT_ps, mvT=mvT)

        def phase_B0():
            """Load the replicated batch-0 rank row and broadcast it."""
            n0h = wpool.tile([1, 512], F16, name="n0h", bufs=1)
            nc.sync.dma_start(n0h[:], n0h_d.ap())
            n0hB = cpool.tile([128, 512], F16, name="n0hB")
            nc.gpsimd.partition_broadcast(n0hB[:], n0h[:])
            st["n0hB"] = n0hB

        def phase_B(s):
            """Ranks, softmax, rank-match scatter, circulant lhs roundtrip launch."""
            phase_Bhead(s)
            d = st[s]
            n0hB = st["n0hB"]
            rs = wpool.tile([128, 4], F32, name=f"rs_{s}", tag="rs", bufs=2)
            for j in range(4):
                c2 = wpool.tile([128, 512], F32R, name=f"c2_{s}_{j}", tag="c2", bufs=2)
                nc.vector.tensor_scalar(c2[:], d["mvB_ps"][:], d["mvT"][:, j:j + 1], 0.0, AL.is_lt,
                                        AL.add, accum_out=rs[:, j:j + 1])
            rsm = wpool.tile([128, 4], F32, name=f"rsm_{s}", tag="rsm", bufs=2)
            nc.vector.tensor_scalar(rsm[:], rs[:], -255.5, None, AL.add)

            smc = wpool.tile([128, 4], BF16, name=f"smc_{s}", tag="smc", bufs=2)
            nc.scalar.activation(smc[:], d["mvT_ps"][:], AF.Exp)
            expz = wpool.tile([1, 512], F32, name=f"expz_{s}", tag="expz", bufs=2)
            z_sb = wpool.tile([1, 1], F32, name=f"z_{s}", tag="z", bufs=2)
            nc.scalar.activation(expz[:], d["mv_sb"][:], AF.Exp, accum_out=z_sb[:])
            rz = wpool.tile([1, 1], F32, name=f"rz_{s}", tag="rz", bufs=2)
            nc.vector.reciprocal(rz[:], z_sb[:])

            g_ps = psmall.tile([1, 512], F32, name=f"gps_{s}", tag="psm", bufs=2)
            for j in range(4):
                wt = wpool.tile([128, 512], BF16, name=f"wt_{s}_{j}", tag="wt", bufs=2)
                nc.vector.tensor_scalar(wt[:], n0hB[:], rsm[:, j:j + 1], None, AL.is_equal)
                nc.tensor.matmul(g_ps[:], smc[:, j:j + 1], wt[:], start=(j == 0), stop=(j == 3))
            gn = wpool.tile([1, 512], BF16, name=f"gn_{s}", tag="gn", bufs=2)
            nc.scalar.activation(gn[:], g_ps[:], AF.Copy, bias=0.0, scale=rz[:])

            gB = wpool.tile([128, 512], BF16, name=f"gB_{s}", tag="gB", bufs=2)
            nc.gpsimd.partition_broadcast(gB[:], gn[:])
            Gb = dpool.tile([128, 1024], BF16, name=f"Gb_{s}", tag="Gb", bufs=2)
            src = bass.AP(tensor=gB[:].tensor, offset=gB[:].offset,
                          ap=[[gB[:].ap[0][0], 128], [0, 2], [1, 512]])
            dst = bass.AP(tensor=Gb[:].tensor, offset=0, ap=[[1024, 128], [512, 2], [1, 512]])
            nc.sync.dma_start(dst, src)
            # T[p, 128*mm + t'] = g_rev[(511 - p + 128*mm + t') % 512]
            T = wpool.tile([128, 512], BF16, name=f"T_{s}", tag="T", bufs=2)
            srcT = bass.AP(tensor=Gb[:].tensor, offset=511,
                           ap=[[1023, 128], [128, 4], [1, 128]])
            dstT = bass.AP(tensor=T[:].tensor, offset=T[:].offset,
                           ap=[[T[:].ap[0][0], 128], [128, 4], [1, 128]])
            nc.sync.dma_start(dstT, srcT)
            # v load for stage C
            vt = iopool.tile([128, 2048], BF16, name=f"v_s{s}", tag="vt", bufs=2)
            src = bass.AP(tensor=v_all, offset=s * L * C,
                          ap=[[512, 128], [65536, 4], [1, 512]])
            dst = bass.AP(tensor=vt[:].tensor, offset=vt[:].offset,
                          ap=[[vt[:].ap[0][0], 128], [512, 4], [1, 512]])
            nc.sync.dma_start(dst, src)
            st[s].update(T=T, vt=vt)

        def phase_C(s):
            """Circulant aggregation + output."""
            T, vt = st[s]["T"], st[s]["vt"]
            osb = wpool.tile([128, 2048], BF16, name=f"osb_{s}", tag="osb", bufs=2)
            for tt in range(4):
                o_ps = pbig.tile([128, 512], F32, name=f"ops_{s}_{tt}", tag="ops", bufs=2)
                for ss in range(4):
                    mm = (tt - ss) % 4
                    nc.tensor.matmul(o_ps[:], T[:, 128 * mm:128 * (mm + 1)],
                                     vt[:, 512 * ss:512 * (ss + 1)],
                                     start=(ss == 0), stop=(ss == 3))
                if tt % 2 == 0:
                    nc.scalar.copy(osb[:, 512 * tt:512 * (tt + 1)], o_ps[:])
                else:
                    nc.vector.tensor_copy(osb[:, 512 * tt:512 * (tt + 1)], o_ps[:])
            dsto = bass.AP(tensor=out_all, offset=s * L * C,
                           ap=[[512, 128], [65536, 4], [1, 512]])
            srco = bass.AP(tensor=osb[:].tensor, offset=osb[:].offset,
                           ap=[[osb[:].ap[0][0], 128], [512, 4], [1, 512]])
            nc.sync.dma_start(dsto, srco)

        # software-pipelined emission
        import os as _os
        _order = _os.environ.get("EMIT", "Z;A0;B0;A1;B1;A2;B2;C0;A3;B3;C1;C2;C3")
        for tok in _order.split(";"):
            if tok == "Z":
                phase_B0()
            elif tok[0] == "A":
                phase_A(int(tok[1]))
            elif tok[0] == "B":
                phase_B(int(tok[1]))
            elif tok[0] == "C":
                phase_C(int(tok[1]))

    nc.compile()
    _NC_CACHE = nc
    return nc


def kernel(queries, keys, values):
    q = np.ascontiguousarray(queries, dtype=np.float32).reshape(B, L, C)
    k = np.ascontiguousarray(keys, dtype=np.float32).reshape(B, L, C)
    v = np.ascontiguousarray(values, dtype=np.float32).reshape(B, L, C)
    qT = np.ascontiguousarray(q.transpose(0, 2, 1)).astype(ml_dtypes.bfloat16)
    kT = np.ascontiguousarray(k.transpose(0, 2, 1)).astype(ml_dtypes.bfloat16)
    vb = v.astype(ml_dtypes.bfloat16)
    # batch-0 delay ranks ("replicate that row" sharding): mv0 once via FFT,
    # rank row in the kernel's reversed coordinates, replicated to every core.
    q0f = np.fft.rfft(q[0], axis=0)          # [F, C] over time axis
    k0f = np.fft.rfft(k[0], axis=0)
    corr0 = np.fft.irfft(q0f * np.conj(k0f), n=L, axis=0)  # [L, C]
    mv0 = corr0.mean(axis=1)                 # [L]
    nless0 = (mv0[None, :] < mv0[:, None]).sum(axis=1).astype(np.float32)
    n0h_rev = (nless0[::-1] - 255.5).astype(np.float16)[None, :]
    nc = _build()
    in_maps = []
    for c in range(NCORES):
        sl = slice(NB * c, NB * (c + 1))
        in_maps.append({
            "qT_all": qT[sl],
            "kT_all": kT[sl],
            "v_all": vb[sl],
            "n0h_in": n0h_rev,
        })
    res = run_bass_kernel_spmd(nc, in_maps, core_ids=list(range(NCORES)))
    out = np.concatenate([res.results[c]["out_all"].astype(np.float32) for c in range(NCORES)], axis=0)
    return out.reshape(B, L, H, E)


if __name__ == "__main__":
    rng = np.random.default_rng(0)
    qq = rng.standard_normal((B, L, H, E)).astype(np.float32)
    kk = rng.standard_normal((B, L, H, E)).astype(np.float32)
    vv = rng.standard_normal((B, L, H, E)).astype(np.float32)
    o = kernel(queries=qq, keys=kk, values=vv)
    print(o.shape, o.dtype, np.abs(o).max())
